# revision 5
# baseline (speedup 1.0000x reference)
"""Trainium2 Bass kernel for nn_NewtonLoss (segment_reduce).

Computes, for K refinement states over N atoms grouped into M molecules:
    sq[k,i]   = ||states_x[k,i,:] - x_target[i,:]||^2
    S[m,k]    = segment_sum(sq[k], molecule_id)
    per_state = sum_m valid_m * S[m,k]/c_m / V
    loss      = sum_k w_k * per_state_k        (w = normalized gamma powers)

Strategy (8-core SPMD, memory-bound):
  - Atoms are sharded across 8 NeuronCores as overlapping fixed-size
    windows; each atom is OWNED by exactly one core (host picks windows).
  - On device, per core: stream states (fp32->fp16 cast on DMA), subtract
    target, square on the scalar engine, reduce xyz, then run a masked
    segmented prefix-scan along atoms (state = mask*state + sq) so the
    scan value at the last atom of each molecule-piece is that piece's
    sum. A fused multiply-reduce against a host-built sparse weight
    vector (1/molecule_count at piece ends, 0 elsewhere) yields per-core
    partial sums of S[m,k]/c_m.
  - Host combines the 8 tiny partial vectors into the final scalar.

The only host-side math on the big arrays is building the boundary mask
and the piece-end weight vector from molecule_id (pure index metadata,
per the molecule-contiguous sharding hint); all floating-point work on
states/target runs on the NeuronCores.
"""

import os
import sys

import numpy as np

for _p in ("/opt/trn_rl_repo",):
    if os.path.isdir(_p) and _p not in sys.path:
        sys.path.insert(0, _p)

import concourse.bacc as bacc  # noqa: E402
import concourse.bass as bass  # noqa: E402
import concourse.tile as tile  # noqa: E402
from concourse import mybir  # noqa: E402

GAMMA = 0.7
NCORES = 8
P = 128  # partitions

# Full-problem geometry (N = 2_000_000 atoms):
#   per-core window = NTILES * P * R atoms; overlapping windows cover N.
K_FULL = 8
R_FULL = 128          # atoms per partition-row per tile
NTILES_FULL = 16
CHUNK_TILES_FULL = 4  # tiles per scan chunk

# "sq" variant geometry: SHARD = 8*128*245 = 250_880 (0.35% window overlap)
R_SQ = 245
NTILES_SQ = 8

DEFAULT_VARIANT = "scan"


def build_program_sq(K=K_FULL, ntiles=NTILES_SQ, R=R_SQ, reps=1,
                     stbufs=3, dbufs=2, d2bufs=2, num_devices=1):
    """Per-atom-weight formulation: loss decomposes as
        sum_k w_k/V * sum_i a_i ||s_ki - t_i||^2,  a_i = owned_i / count[mol(i)]
    so no segmented scan is needed. Host ships sqrt(a_i) replicated x3 as
    f16 ("wvec"); device computes d2 = (s - t)*sqrt(a) on DVE (two 2x f16
    tensor_tensor ops) and Square+accum on the scalar engine per (tile, k).
    """
    TILE = P * R
    SHARD = ntiles * TILE
    RD = R * 3
    f32 = mybir.dt.float32
    f16 = mybir.dt.float16

    nc = bacc.Bacc("TRN2", target_bir_lowering=False, debug=False,
                   num_devices=num_devices)
    states = nc.dram_tensor("states", [K, SHARD, 3], f32, kind="ExternalInput").ap()
    target = nc.dram_tensor("target", [SHARD, 3], f32, kind="ExternalInput").ap()
    wvecd = nc.dram_tensor("wvec", [SHARD, 3], f16, kind="ExternalInput").ap()
    accd = nc.dram_tensor("acc", [P, ntiles * K], f32, kind="ExternalOutput").ap()

    # atom i lives at (tile t, partition p, row-pos r): i = t*TILE + p*R + r
    st_v = states.rearrange("k (t p r) d -> t p k (r d)", t=ntiles, p=P)
    tg_v = target.rearrange("(t p r) d -> t p (r d)", t=ntiles, p=P)
    wv_v = wvecd.rearrange("(t p r) d -> t p (r d)", t=ntiles, p=P)

    def bcast_k(ap2d):
        # [P, RD] -> [P, (0,K), RD]: replicate a per-atom row over the K dim
        return bass.AP(tensor=ap2d.tensor, offset=ap2d.offset,
                       ap=[list(ap2d.ap[0]), [0, K], list(ap2d.ap[-1])])

    with tile.TileContext(nc) as tc:
        with (
            tc.tile_pool(name="singles", bufs=1) as singles,
            tc.tile_pool(name="tgp", bufs=1) as tgp,
            tc.tile_pool(name="wvp", bufs=1) as wvp,
            tc.tile_pool(name="stp", bufs=stbufs) as stp,
            tc.tile_pool(name="dfp", bufs=dbufs) as dfp,
            tc.tile_pool(name="d2p", bufs=d2bufs) as d2p,
        ):
            acc = singles.tile([P, ntiles * K], f32)
            # per-tile slices of target/weights so tile 0 compute starts early
            tg_all = tgp.tile([P, ntiles, RD], f16, bufs=1)
            wv_all = wvp.tile([P, ntiles, RD], f16, bufs=1)
            for t in range(ntiles):
                nc.gpsimd.dma_start(out=tg_all[:, t, :], in_=tg_v[t])  # f32->f16
                nc.sync.dma_start(out=wv_all[:, t, :], in_=wv_v[t])
            for _rep in range(reps):
                for t in range(ntiles):
                    st = stp.tile([P, K, RD], f16)
                    nc.gpsimd.dma_start(out=st, in_=st_v[t])  # f32->f16 cast
                    diff = dfp.tile([P, K, RD], f16)
                    nc.vector.tensor_sub(diff, st, bcast_k(tg_all[:, t, :]))
                    d2 = d2p.tile([P, K, RD], f16)
                    nc.vector.tensor_mul(d2, diff, bcast_k(wv_all[:, t, :]))
                    for k in range(K):
                        slot = acc[:, t * K + k: t * K + k + 1]
                        nc.scalar.activation(
                            d2[:, k, :], d2[:, k, :],
                            mybir.ActivationFunctionType.Square,
                            accum_out=slot)
            nc.sync.dma_start(out=accd, in_=acc)
    nc.compile()
    return nc


def build_program(K=K_FULL, ntiles=NTILES_FULL, R=R_FULL,
                  chunk_tiles=CHUNK_TILES_FULL, reps=1, variant="scan",
                  add1_engine="gpsimd", add2_engine="vector",
                  red_mode="act", stbufs=3, chbufs=2,
                  scan_engine="vector", mul_engine="vector",
                  stop_after="full", cast=True, dropmask=False,
                  num_devices=1):
    """Build the single-core Bass program (run SPMD on all cores).

    variant "scan": masked segmented scan + piece-end weights (W sparse).
    variant "uw":   per-atom 1/count weights, no scan (W dense).
    red_mode "ttr": fused multiply-reduce on DVE.
    red_mode "act": multiply on DVE, accumulate via ACT Copy(accum_out).
    """
    TILE = P * R
    SHARD = ntiles * TILE
    RD = R * 3
    nchunks = ntiles // chunk_tiles
    CH = chunk_tiles * R  # scan length per chunk per partition
    f32 = mybir.dt.float32
    f16 = mybir.dt.float16 if cast else f32
    add, mult = mybir.AluOpType.add, mybir.AluOpType.mult

    nc = bacc.Bacc("TRN2", target_bir_lowering=False, debug=False,
                   num_devices=num_devices)
    states = nc.dram_tensor("states", [K, SHARD, 3], f32, kind="ExternalInput").ap()
    target = nc.dram_tensor("target", [SHARD, 3], f32, kind="ExternalInput").ap()
    maskd = (None if dropmask else
             nc.dram_tensor("mask", [SHARD], f32, kind="ExternalInput").ap())
    wvecd = nc.dram_tensor("wvec", [SHARD], f32, kind="ExternalInput").ap()
    accd = nc.dram_tensor("acc", [P, nchunks * K], f32, kind="ExternalOutput").ap()

    # atom i lives at (tile t, partition p, row-pos r): i = t*TILE + p*R + r
    st_v = states.rearrange("k (t p r) d -> t p k (r d)", t=ntiles, p=P)
    tg_v = target.rearrange("(t p r) d -> p t (r d)", t=ntiles, p=P)
    mk_v = (None if dropmask else
            maskd.rearrange("(t p r) -> p t r", t=ntiles, p=P))
    wv_v = wvecd.rearrange("(t p r) -> p t r", t=ntiles, p=P)

    engines = {"vector": nc.vector, "gpsimd": nc.gpsimd}
    add1_e, add2_e = engines[add1_engine], engines[add2_engine]
    mul_e = engines[mul_engine]

    def scan_e(k):
        if scan_engine == "split":
            return nc.vector if k % 2 == 0 else nc.gpsimd
        return engines[scan_engine]

    with tile.TileContext(nc) as tc:
        with (
            tc.tile_pool(name="singles", bufs=1) as singles,
            tc.tile_pool(name="stp", bufs=stbufs) as stp,
            tc.tile_pool(name="dfp", bufs=2) as dfp,
            tc.tile_pool(name="sqp", bufs=2) as sqp,
            tc.tile_pool(name="tmpp", bufs=2) as tmpp,
            tc.tile_pool(name="chp", bufs=chbufs) as chp,
            tc.tile_pool(name="scp", bufs=2) as scp,
            tc.tile_pool(name="ttp", bufs=2) as ttp,
        ):
            tg_all = singles.tile([P, ntiles, RD], f16)
            (nc.gpsimd if cast else nc.sync).dma_start(out=tg_all, in_=tg_v)
            wv_all = singles.tile([P, ntiles, R], f32)
            nc.sync.dma_start(out=wv_all, in_=wv_v)
            if not dropmask:
                # load mask even if unused: a declared-but-stripped input
                # tensor crashes the pjrt exec path
                mk_all = singles.tile([P, ntiles, R], f32)
                nc.sync.dma_start(out=mk_all, in_=mk_v)
            acc = singles.tile([P, nchunks * K], f32)
            order = ["dma", "sub", "sq", "adds", "scan", "full"]
            lvl = order.index(stop_after)
            if lvl < 5:
                nc.vector.memset(acc, 0.0)

            for _rep in range(reps):
                for ch in range(nchunks):
                    sqbuf = chp.tile([P, K, chunk_tiles, R], f32)
                    for j in range(chunk_tiles):
                        t = ch * chunk_tiles + j
                        st = stp.tile([P, K, RD], f16)
                        (nc.gpsimd if cast else nc.sync).dma_start(
                            out=st, in_=st_v[t])
                        if lvl < 1:
                            continue
                        diff = dfp.tile([P, K, RD], f16)
                        tgs = tg_all[:, t, :]
                        tgb = bass.AP(
                            tensor=tgs.tensor, offset=tgs.offset,
                            ap=[list(tgs.ap[0]), [0, K], list(tgs.ap[-1])],
                        )
                        nc.vector.tensor_sub(diff, st, tgb)
                        if lvl < 2:
                            continue
                        sq = sqp.tile([P, K, RD], f32)
                        nc.scalar.square(sq, diff)
                        if lvl < 3:
                            continue
                        sq4 = sq.rearrange("p k (r d) -> p k r d", d=3)
                        tmp = tmpp.tile([P, K, R], f32)
                        add1_e.tensor_add(tmp, sq4[:, :, :, 0], sq4[:, :, :, 1])
                        add2_e.tensor_add(sqbuf[:, :, j, :], tmp, sq4[:, :, :, 2])
                    if lvl < 4:
                        continue
                    wv_ch = wv_all[:, ch * chunk_tiles:(ch + 1) * chunk_tiles, :]
                    wv_ch = wv_ch.rearrange("p t r -> p (t r)")
                    if variant == "scan":
                        mk_ch = mk_all[:, ch * chunk_tiles:(ch + 1) * chunk_tiles, :]
                        mk_ch = mk_ch.rearrange("p t r -> p (t r)")
                    for k in range(K):
                        red_in = sqbuf[:, k, :, :].rearrange("p t r -> p (t r)")
                        if variant == "scan":
                            scano = scp.tile([P, CH], f32)
                            scan_e(k).tensor_tensor_scan(
                                out=scano, data0=mk_ch, data1=red_in,
                                initial=0.0, op0=mult, op1=add)
                            red_in = scano
                        if lvl < 5:
                            continue
                        tto = ttp.tile([P, CH], f32)
                        acc_slot = acc[:, ch * K + k: ch * K + k + 1]
                        if red_mode == "stt":
                            mul_e.scalar_tensor_tensor(
                                out=tto, in0=red_in, scalar=1.0, in1=wv_ch,
                                op0=mult, op1=mult, accum_out=acc_slot)
                        elif red_mode == "act":
                            mul_e.tensor_mul(tto, red_in, wv_ch)
                            nc.scalar.activation(
                                tto, tto, mybir.ActivationFunctionType.Copy,
                                accum_out=acc_slot)
                        else:
                            nc.vector.tensor_tensor_reduce(
                                out=tto, in0=red_in, in1=wv_ch, scale=1.0,
                                scalar=0.0, op0=mult, op1=add,
                                accum_out=acc_slot)
            nc.sync.dma_start(out=accd, in_=acc)
    nc.compile()
    return nc


def variant_geom(variant, ntiles=None, R=None):
    if ntiles is None:
        ntiles = NTILES_SQ if variant == "sq" else NTILES_FULL
    if R is None:
        R = R_SQ if variant == "sq" else R_FULL
    return ntiles, R


def host_prep(states_x, x_target, molecule_id, num_molecules,
              ncores=NCORES, K=K_FULL, ntiles=None, R=None,
              variant="scan"):
    """Shard inputs into per-core windows; build mask/weight vectors.

    Returns (in_maps, V) where in_maps[c] are the named inputs for core c.
    """
    ntiles, R = variant_geom(variant, ntiles, R)
    TILE = P * R
    SHARD = ntiles * TILE
    N = molecule_id.shape[0]
    M = int(num_molecules)
    assert N % ncores == 0
    OWN = N // ncores
    assert SHARD >= OWN, (SHARD, OWN)

    ids = np.asarray(molecule_id).astype(np.int64)
    counts = np.bincount(ids, minlength=M)
    V = int((counts > 0).sum())
    inv_c = np.zeros(M, np.float64)
    nz = counts > 0
    inv_c[nz] = 1.0 / counts[nz]

    states_x = np.asarray(states_x)
    x_target = np.asarray(x_target)

    r_idx = np.arange(SHARD, dtype=np.int64) % R

    in_maps = []
    for c in range(ncores):
        S_c = 0 if ncores == 1 else (c * (N - SHARD)) // (ncores - 1)
        own_lo, own_hi = c * OWN - S_c, (c + 1) * OWN - S_c
        assert own_lo >= 0 and own_hi <= SHARD

        idw = ids[S_c:S_c + SHARD]
        pos = np.arange(SHARD, dtype=np.int64)
        owned = (pos >= own_lo) & (pos < own_hi)

        if variant == "sq":
            w = np.where(owned, inv_c[idw], 0.0)
            wv3 = np.repeat(np.sqrt(w).astype(np.float16)[:, None], 3, axis=1)
            in_maps.append({
                "states": np.ascontiguousarray(
                    states_x[:, S_c:S_c + SHARD, :], dtype=np.float32),
                "target": np.ascontiguousarray(
                    x_target[S_c:S_c + SHARD, :], dtype=np.float32),
                "wvec": np.ascontiguousarray(wv3),
            })
            continue
        if variant == "uw":
            m = np.zeros(SHARD, np.float32)
            w = np.where(owned, inv_c[idw], 0.0)
        else:
            same_prev = np.zeros(SHARD, bool)
            same_prev[1:] = idw[1:] == idw[:-1]
            m = (r_idx > 0) & same_prev & owned
            m[1:] &= owned[:-1]

            nxt_same = np.zeros(SHARD, bool)
            nxt_same[:-1] = idw[:-1] == idw[1:]
            nxt_same[:-1] &= owned[1:]
            nxt_same &= r_idx < (R - 1)
            w = np.where(owned & ~nxt_same, inv_c[idw], 0.0)

        in_maps.append({
            "states": np.ascontiguousarray(states_x[:, S_c:S_c + SHARD, :],
                                           dtype=np.float32),
            "target": np.ascontiguousarray(x_target[S_c:S_c + SHARD, :],
                                           dtype=np.float32),
            "mask": np.asarray(m, np.float32),
            "wvec": np.asarray(w, np.float32),
        })
    return in_maps, V


def combine(results, V, K=K_FULL):
    """Sum per-core accumulators into the final scalar loss."""
    total = np.zeros(K, np.float64)
    for r in results:
        acc = np.asarray(r["acc"]).astype(np.float64)  # [P, nchunks*K]
        total += acc.reshape(P, -1, K).sum(axis=(0, 1))
    per_state = total / V
    w = GAMMA ** ((K - 1) - np.arange(K, dtype=np.float64))
    w = w / w.sum()
    return np.float32((w * per_state).sum())


class Runner:
    """Caches the compiled PJRT executable for repeated SPMD runs."""

    def __init__(self, nc, n_cores=NCORES, n_inner=1):
        import jax
        from jax.experimental.shard_map import shard_map
        from jax.sharding import Mesh, PartitionSpec
        from concourse import bass2jax

        bass2jax.install_neuronx_cc_hook()
        self.jax = jax
        self.nc = nc
        self.n_cores = n_cores

        partition_name = (nc.partition_id_tensor.name
                          if nc.partition_id_tensor else None)
        in_names, out_names, out_avals, zero_outs = [], [], [], []
        for alloc in nc.m.functions[0].allocations:
            if not isinstance(alloc, mybir.MemoryLocationSet):
                continue
            name = alloc.memorylocations[0].name
            if alloc.kind == "ExternalInput":
                if name != partition_name:
                    in_names.append(name)
            elif alloc.kind == "ExternalOutput":
                shape = tuple(alloc.tensor_shape)
                dtype = mybir.dt.np(alloc.dtype)
                out_names.append(name)
                out_avals.append(jax.core.ShapedArray(shape, dtype))
                zero_outs.append(np.zeros(shape, dtype))
        self.in_names, self.out_names = in_names, out_names
        self.out_avals, self.zero_outs = out_avals, zero_outs
        n_params = len(in_names)
        all_in_names = list(in_names) + list(out_names)
        if partition_name is not None:
            all_in_names.append(partition_name)

        def _body(*args):
            ins = list(args[:n_params])
            cur_zeros = list(args[n_params:n_params + len(out_names)])
            extra = ([bass2jax.partition_id_tensor()]
                     if partition_name is not None else [])
            outs = tuple(cur_zeros)
            for _ in range(n_inner):
                # chain outputs into the next call's output buffers: keeps
                # every invocation live (no CSE/DCE) and is a no-op since
                # the kernel fully overwrites its outputs
                outs = bass2jax._bass_exec_p.bind(
                    *ins, *outs, *extra,
                    out_avals=tuple(out_avals),
                    in_names=tuple(all_in_names),
                    out_names=tuple(out_names),
                    lowering_input_output_aliases=(),
                    sim_require_finite=True,
                    sim_require_nnan=True,
                    nc=nc,
                )
            return tuple(outs)

        devices = jax.devices()[:n_cores]
        assert len(devices) == n_cores
        self.mesh = Mesh(np.asarray(devices), ("core",))
        self.pspec = PartitionSpec("core")
        n_outs = len(out_names)
        in_specs = (self.pspec,) * (n_params + n_outs)
        out_specs = (self.pspec,) * n_outs
        donate = tuple(range(n_params, n_params + n_outs))
        self.fn = jax.jit(
            shard_map(_body, mesh=self.mesh, in_specs=in_specs,
                      out_specs=out_specs, check_rep=False),
            donate_argnums=donate, keep_unused=True)

    def concat_inputs(self, in_maps):
        return [np.concatenate([np.asarray(in_maps[c][n])
                                for c in range(self.n_cores)], axis=0)
                for n in self.in_names]

    def device_put(self, concat_in):
        from jax.sharding import NamedSharding
        sh = NamedSharding(self.mesh, self.pspec)
        return [self.jax.device_put(a, sh) for a in concat_in]

    def run_dev(self, dev_args):
        zeros = [np.zeros((self.n_cores * z.shape[0], *z.shape[1:]), z.dtype)
                 for z in self.zero_outs]
        out = self.fn(*dev_args, *zeros)
        return self.jax.block_until_ready(out)

    def run(self, in_maps):
        out_arrs = self.run_dev(self.device_put(self.concat_inputs(in_maps)))
        return [
            {n: np.asarray(out_arrs[i]).reshape(
                self.n_cores, *self.out_avals[i].shape)[c]
             for i, n in enumerate(self.out_names)}
            for c in range(self.n_cores)
        ]


_CACHE = {}


def get_runner(variant=DEFAULT_VARIANT, reps=1, n_inner=1, **kw):
    key = (variant, reps, n_inner, tuple(sorted(kw.items())))
    if key not in _CACHE:
        if variant == "sq":
            nc = build_program_sq(reps=reps, **kw)
        else:
            nc = build_program(variant=variant, reps=reps, **kw)
        _CACHE[key] = Runner(nc, n_inner=n_inner)
    return _CACHE[key]


def kernel(states_x, x_target, molecule_id, num_molecules):
    runner = get_runner(DEFAULT_VARIANT)
    in_maps, V = host_prep(states_x, x_target, molecule_id, num_molecules,
                           variant=DEFAULT_VARIANT)
    results = runner.run(in_maps)
    return combine(results, V)



# revision 14
# speedup vs baseline: 5.5645x; 5.5645x over previous
"""Trainium2 Bass kernel for nn_NewtonLoss (segment_reduce).

Computes, for K refinement states over N atoms grouped into M molecules:
    sq[k,i]   = ||states_x[k,i,:] - x_target[i,:]||^2
    S[m,k]    = segment_sum(sq[k], molecule_id)
    per_state = sum_m valid_m * S[m,k]/c_m / V
    loss      = sum_k w_k * per_state_k        (w = normalized gamma powers)

Strategy (8-core SPMD, memory-bound):
  - Atoms are sharded across 8 NeuronCores as overlapping fixed-size
    windows; each atom is OWNED by exactly one core (host picks windows).
  - On device, per core: stream states (fp32->fp16 cast on DMA), subtract
    target, square on the scalar engine, reduce xyz, then run a masked
    segmented prefix-scan along atoms (state = mask*state + sq) so the
    scan value at the last atom of each molecule-piece is that piece's
    sum. A fused multiply-reduce against a host-built sparse weight
    vector (1/molecule_count at piece ends, 0 elsewhere) yields per-core
    partial sums of S[m,k]/c_m.
  - Host combines the 8 tiny partial vectors into the final scalar.

The only host-side math on the big arrays is building the boundary mask
and the piece-end weight vector from molecule_id (pure index metadata,
per the molecule-contiguous sharding hint); all floating-point work on
states/target runs on the NeuronCores.
"""

import os
import sys

import numpy as np

for _p in ("/opt/trn_rl_repo",):
    if os.path.isdir(_p) and _p not in sys.path:
        sys.path.insert(0, _p)

import concourse.bacc as bacc  # noqa: E402
import concourse.bass as bass  # noqa: E402
import concourse.tile as tile  # noqa: E402
from concourse import mybir  # noqa: E402

GAMMA = 0.7
NCORES = 8
P = 128  # partitions

# Full-problem geometry (N = 2_000_000 atoms):
#   per-core window = NTILES * P * R atoms; overlapping windows cover N.
K_FULL = 8
R_FULL = 128          # atoms per partition-row per tile
NTILES_FULL = 16
CHUNK_TILES_FULL = 4  # tiles per scan chunk

# "sq" variant geometry: SHARD = 128*1960 = 250_880 (0.35% window overlap)
R_SQ = 245
NTILES_SQ = 8
SHARD_SQ = 250_880
# variable tile sizes: big tiles stream first; small final tiles shrink the
# post-DMA tail (last tile's DVE + last ACT group). sum(R_LIST_SQ)*128 = SHARD
R_LIST_SQ = (245, 245, 245, 245, 245, 245, 245, 245)
# ACT accumulation groups over tile indices (amortize ScalarE fixed cost)
GROUPS_SQ = ((0, 1), (2, 3), (4, 5), (6, 7))

DEFAULT_VARIANT = "scan"


def build_program_sq(K=K_FULL, r_list=R_LIST_SQ, groups=GROUPS_SQ, reps=1,
                     stbufs=4, dbufs=2, d2bufs=2, num_devices=1):
    """Per-atom-weight formulation: loss decomposes as
        sum_k w_k/V * sum_i a_i ||s_ki - t_i||^2,  a_i = owned_i / count[mol(i)]
    so no segmented scan is needed. Host ships sqrt(a_i) replicated x3 as
    f16 ("wvec"); device computes d2 = (s - t)*sqrt(a) on DVE (two 2x f16
    tensor_tensor ops) per tile, then the scalar engine does Square with
    accum_out per (tile-group, k). Tile t covers P*r_list[t] atoms; groups
    partition the tile indices for ACT accumulation. Big tiles stream first
    (DMA stays saturated); small final tiles shrink the post-DMA tail.
    """
    SHARD = P * sum(r_list)
    assert SHARD == SHARD_SQ, SHARD
    ntiles = len(r_list)
    assert tuple(sorted(t for gr in groups for t in gr)) == tuple(range(ntiles))
    f32 = mybir.dt.float32
    f16 = mybir.dt.float16
    ngroups = len(groups)

    nc = bacc.Bacc("TRN2", target_bir_lowering=False, debug=False,
                   num_devices=num_devices)
    states = nc.dram_tensor("states", [K, SHARD, 3], f32, kind="ExternalInput").ap()
    target = nc.dram_tensor("target", [SHARD, 3], f32, kind="ExternalInput").ap()
    wvecd = nc.dram_tensor("wvec", [SHARD, 3], f16, kind="ExternalInput").ap()
    accd = nc.dram_tensor("acc", [P, ngroups * K], f32, kind="ExternalOutput").ap()

    # atom i of tile t lives at (partition p, row-pos r): i = lo_t + p*R_t + r
    lo = [P * sum(r_list[:t]) for t in range(ntiles)]
    st_v, tg_v, wv_v = [], [], []
    for t, R in enumerate(r_list):
        sl = slice(lo[t], lo[t] + P * R)
        st_v.append(states[:, sl, :].rearrange("k (p r) d -> p k (r d)", p=P))
        tg_v.append(target[sl, :].rearrange("(p r) d -> p (r d)", p=P))
        wv_v.append(wvecd[sl, :].rearrange("(p r) d -> p (r d)", p=P))

    def bcast_k(ap2d):
        # [P, RD] -> [P, (0,K), RD]: replicate a per-atom row over the K dim
        return bass.AP(tensor=ap2d.tensor, offset=ap2d.offset,
                       ap=[list(ap2d.ap[0]), [0, K], list(ap2d.ap[-1])])

    with tile.TileContext(nc) as tc:
        with (
            tc.tile_pool(name="singles", bufs=1) as singles,
            tc.tile_pool(name="tgp", bufs=1) as tgp,
            tc.tile_pool(name="wvp", bufs=1) as wvp,
            tc.tile_pool(name="stp", bufs=stbufs) as stp,
            tc.tile_pool(name="dfp", bufs=dbufs) as dfp,
            tc.tile_pool(name="d2p", bufs=d2bufs) as d2p,
        ):
            acc = singles.tile([P, ngroups * K], f32)
            RDmax = 3 * max(r_list)
            # per-tile target/weight tiles (uniform max size, sliced per use)
            tg_all = tgp.tile([P, ntiles, RDmax], f16, bufs=1)
            wv_all = wvp.tile([P, ntiles, RDmax], f16, bufs=1)
            for t, R in enumerate(r_list):
                nc.sync.dma_start(out=wv_all[:, t, :3 * R], in_=wv_v[t])
            GRD = {g: 3 * sum(r_list[t] for t in gr)
                   for g, gr in enumerate(groups)}
            for _rep in range(reps):
                for g, gr in enumerate(groups):
                    d2 = d2p.tile([P, K, GRD[g]], f16, tag="d2")
                    off = 0
                    for t in gr:
                        R = r_list[t]
                        RD = 3 * R
                        st = stp.tile([P, K, RD], f16, tag="st")
                        if _rep == 0:
                            # tile t's target lands just ahead of its states
                            # (same SWDGE queue)
                            nc.gpsimd.dma_start(out=tg_all[:, t, :RD],
                                                in_=tg_v[t])
                        nc.gpsimd.dma_start(out=st, in_=st_v[t])  # f32->f16
                        diff = dfp.tile([P, K, RD], f16, tag="diff")
                        nc.vector.tensor_sub(diff, st,
                                             bcast_k(tg_all[:, t, :RD]))
                        nc.vector.tensor_mul(d2[:, :, off:off + RD], diff,
                                             bcast_k(wv_all[:, t, :RD]))
                        off += RD
                    for k in range(K):
                        slot = acc[:, g * K + k: g * K + k + 1]
                        nc.scalar.activation(
                            d2[:, k, :], d2[:, k, :],
                            mybir.ActivationFunctionType.Square,
                            accum_out=slot)
            nc.sync.dma_start(out=accd, in_=acc)
    nc.compile()
    return nc


def build_program(K=K_FULL, ntiles=NTILES_FULL, R=R_FULL,
                  chunk_tiles=CHUNK_TILES_FULL, reps=1, variant="scan",
                  add1_engine="gpsimd", add2_engine="vector",
                  red_mode="act", stbufs=3, chbufs=2,
                  scan_engine="vector", mul_engine="vector",
                  stop_after="full", cast=True, dropmask=False,
                  num_devices=1):
    """Build the single-core Bass program (run SPMD on all cores).

    variant "scan": masked segmented scan + piece-end weights (W sparse).
    variant "uw":   per-atom 1/count weights, no scan (W dense).
    red_mode "ttr": fused multiply-reduce on DVE.
    red_mode "act": multiply on DVE, accumulate via ACT Copy(accum_out).
    """
    TILE = P * R
    SHARD = ntiles * TILE
    RD = R * 3
    nchunks = ntiles // chunk_tiles
    CH = chunk_tiles * R  # scan length per chunk per partition
    f32 = mybir.dt.float32
    f16 = mybir.dt.float16 if cast else f32
    add, mult = mybir.AluOpType.add, mybir.AluOpType.mult

    nc = bacc.Bacc("TRN2", target_bir_lowering=False, debug=False,
                   num_devices=num_devices)
    states = nc.dram_tensor("states", [K, SHARD, 3], f32, kind="ExternalInput").ap()
    target = nc.dram_tensor("target", [SHARD, 3], f32, kind="ExternalInput").ap()
    maskd = (None if dropmask else
             nc.dram_tensor("mask", [SHARD], f32, kind="ExternalInput").ap())
    wvecd = nc.dram_tensor("wvec", [SHARD], f32, kind="ExternalInput").ap()
    accd = nc.dram_tensor("acc", [P, nchunks * K], f32, kind="ExternalOutput").ap()

    # atom i lives at (tile t, partition p, row-pos r): i = t*TILE + p*R + r
    st_v = states.rearrange("k (t p r) d -> t p k (r d)", t=ntiles, p=P)
    tg_v = target.rearrange("(t p r) d -> p t (r d)", t=ntiles, p=P)
    mk_v = (None if dropmask else
            maskd.rearrange("(t p r) -> p t r", t=ntiles, p=P))
    wv_v = wvecd.rearrange("(t p r) -> p t r", t=ntiles, p=P)

    engines = {"vector": nc.vector, "gpsimd": nc.gpsimd}
    add1_e, add2_e = engines[add1_engine], engines[add2_engine]
    mul_e = engines[mul_engine]

    def scan_e(k):
        if scan_engine == "split":
            return nc.vector if k % 2 == 0 else nc.gpsimd
        return engines[scan_engine]

    with tile.TileContext(nc) as tc:
        with (
            tc.tile_pool(name="singles", bufs=1) as singles,
            tc.tile_pool(name="stp", bufs=stbufs) as stp,
            tc.tile_pool(name="dfp", bufs=2) as dfp,
            tc.tile_pool(name="sqp", bufs=2) as sqp,
            tc.tile_pool(name="tmpp", bufs=2) as tmpp,
            tc.tile_pool(name="chp", bufs=chbufs) as chp,
            tc.tile_pool(name="scp", bufs=2) as scp,
            tc.tile_pool(name="ttp", bufs=2) as ttp,
        ):
            tg_all = singles.tile([P, ntiles, RD], f16)
            (nc.gpsimd if cast else nc.sync).dma_start(out=tg_all, in_=tg_v)
            wv_all = singles.tile([P, ntiles, R], f32)
            nc.sync.dma_start(out=wv_all, in_=wv_v)
            if not dropmask:
                # load mask even if unused: a declared-but-stripped input
                # tensor crashes the pjrt exec path
                mk_all = singles.tile([P, ntiles, R], f32)
                nc.sync.dma_start(out=mk_all, in_=mk_v)
            acc = singles.tile([P, nchunks * K], f32)
            order = ["dma", "sub", "sq", "adds", "scan", "full"]
            lvl = order.index(stop_after)
            if lvl < 5:
                nc.vector.memset(acc, 0.0)

            for _rep in range(reps):
                for ch in range(nchunks):
                    sqbuf = chp.tile([P, K, chunk_tiles, R], f32)
                    for j in range(chunk_tiles):
                        t = ch * chunk_tiles + j
                        st = stp.tile([P, K, RD], f16)
                        (nc.gpsimd if cast else nc.sync).dma_start(
                            out=st, in_=st_v[t])
                        if lvl < 1:
                            continue
                        diff = dfp.tile([P, K, RD], f16)
                        tgs = tg_all[:, t, :]
                        tgb = bass.AP(
                            tensor=tgs.tensor, offset=tgs.offset,
                            ap=[list(tgs.ap[0]), [0, K], list(tgs.ap[-1])],
                        )
                        nc.vector.tensor_sub(diff, st, tgb)
                        if lvl < 2:
                            continue
                        sq = sqp.tile([P, K, RD], f32)
                        nc.scalar.square(sq, diff)
                        if lvl < 3:
                            continue
                        sq4 = sq.rearrange("p k (r d) -> p k r d", d=3)
                        tmp = tmpp.tile([P, K, R], f32)
                        add1_e.tensor_add(tmp, sq4[:, :, :, 0], sq4[:, :, :, 1])
                        add2_e.tensor_add(sqbuf[:, :, j, :], tmp, sq4[:, :, :, 2])
                    if lvl < 4:
                        continue
                    wv_ch = wv_all[:, ch * chunk_tiles:(ch + 1) * chunk_tiles, :]
                    wv_ch = wv_ch.rearrange("p t r -> p (t r)")
                    if variant == "scan":
                        mk_ch = mk_all[:, ch * chunk_tiles:(ch + 1) * chunk_tiles, :]
                        mk_ch = mk_ch.rearrange("p t r -> p (t r)")
                    for k in range(K):
                        red_in = sqbuf[:, k, :, :].rearrange("p t r -> p (t r)")
                        if variant == "scan":
                            scano = scp.tile([P, CH], f32)
                            scan_e(k).tensor_tensor_scan(
                                out=scano, data0=mk_ch, data1=red_in,
                                initial=0.0, op0=mult, op1=add)
                            red_in = scano
                        if lvl < 5:
                            continue
                        tto = ttp.tile([P, CH], f32)
                        acc_slot = acc[:, ch * K + k: ch * K + k + 1]
                        if red_mode == "stt":
                            mul_e.scalar_tensor_tensor(
                                out=tto, in0=red_in, scalar=1.0, in1=wv_ch,
                                op0=mult, op1=mult, accum_out=acc_slot)
                        elif red_mode == "act":
                            mul_e.tensor_mul(tto, red_in, wv_ch)
                            nc.scalar.activation(
                                tto, tto, mybir.ActivationFunctionType.Copy,
                                accum_out=acc_slot)
                        else:
                            nc.vector.tensor_tensor_reduce(
                                out=tto, in0=red_in, in1=wv_ch, scale=1.0,
                                scalar=0.0, op0=mult, op1=add,
                                accum_out=acc_slot)
            nc.sync.dma_start(out=accd, in_=acc)
    nc.compile()
    return nc


def variant_geom(variant, ntiles=None, R=None):
    if ntiles is None:
        ntiles = NTILES_SQ if variant == "sq" else NTILES_FULL
    if R is None:
        R = R_SQ if variant == "sq" else R_FULL
    return ntiles, R


def host_prep(states_x, x_target, molecule_id, num_molecules,
              ncores=NCORES, K=K_FULL, ntiles=None, R=None,
              variant="scan"):
    """Shard inputs into per-core windows; build mask/weight vectors.

    Returns (in_maps, V) where in_maps[c] are the named inputs for core c.
    """
    ntiles, R = variant_geom(variant, ntiles, R)
    TILE = P * R
    SHARD = ntiles * TILE
    N = molecule_id.shape[0]
    M = int(num_molecules)
    assert N % ncores == 0
    OWN = N // ncores
    assert SHARD >= OWN, (SHARD, OWN)

    ids = np.asarray(molecule_id).astype(np.int64)
    counts = np.bincount(ids, minlength=M)
    V = int((counts > 0).sum())
    inv_c = np.zeros(M, np.float64)
    nz = counts > 0
    inv_c[nz] = 1.0 / counts[nz]

    states_x = np.asarray(states_x)
    x_target = np.asarray(x_target)

    r_idx = np.arange(SHARD, dtype=np.int64) % R

    in_maps = []
    for c in range(ncores):
        S_c = 0 if ncores == 1 else (c * (N - SHARD)) // (ncores - 1)
        own_lo, own_hi = c * OWN - S_c, (c + 1) * OWN - S_c
        assert own_lo >= 0 and own_hi <= SHARD

        idw = ids[S_c:S_c + SHARD]
        pos = np.arange(SHARD, dtype=np.int64)
        owned = (pos >= own_lo) & (pos < own_hi)

        if variant == "sq":
            w = np.where(owned, inv_c[idw], 0.0)
            wv3 = np.repeat(np.sqrt(w).astype(np.float16)[:, None], 3, axis=1)
            in_maps.append({
                "states": np.ascontiguousarray(
                    states_x[:, S_c:S_c + SHARD, :], dtype=np.float32),
                "target": np.ascontiguousarray(
                    x_target[S_c:S_c + SHARD, :], dtype=np.float32),
                "wvec": np.ascontiguousarray(wv3),
            })
            continue
        if variant == "uw":
            m = np.zeros(SHARD, np.float32)
            w = np.where(owned, inv_c[idw], 0.0)
        else:
            same_prev = np.zeros(SHARD, bool)
            same_prev[1:] = idw[1:] == idw[:-1]
            m = (r_idx > 0) & same_prev & owned
            m[1:] &= owned[:-1]

            nxt_same = np.zeros(SHARD, bool)
            nxt_same[:-1] = idw[:-1] == idw[1:]
            nxt_same[:-1] &= owned[1:]
            nxt_same &= r_idx < (R - 1)
            w = np.where(owned & ~nxt_same, inv_c[idw], 0.0)

        in_maps.append({
            "states": np.ascontiguousarray(states_x[:, S_c:S_c + SHARD, :],
                                           dtype=np.float32),
            "target": np.ascontiguousarray(x_target[S_c:S_c + SHARD, :],
                                           dtype=np.float32),
            "mask": np.asarray(m, np.float32),
            "wvec": np.asarray(w, np.float32),
        })
    return in_maps, V


def combine(results, V, K=K_FULL):
    """Sum per-core accumulators into the final scalar loss."""
    total = np.zeros(K, np.float64)
    for r in results:
        acc = np.asarray(r["acc"]).astype(np.float64)  # [P, nchunks*K]
        total += acc.reshape(P, -1, K).sum(axis=(0, 1))
    per_state = total / V
    w = GAMMA ** ((K - 1) - np.arange(K, dtype=np.float64))
    w = w / w.sum()
    return np.float32((w * per_state).sum())


class Runner:
    """Caches the compiled PJRT executable for repeated SPMD runs."""

    def __init__(self, nc, n_cores=NCORES, n_inner=1):
        import jax
        from jax.experimental.shard_map import shard_map
        from jax.sharding import Mesh, PartitionSpec
        from concourse import bass2jax

        bass2jax.install_neuronx_cc_hook()
        self.jax = jax
        self.nc = nc
        self.n_cores = n_cores

        partition_name = (nc.partition_id_tensor.name
                          if nc.partition_id_tensor else None)
        in_names, out_names, out_avals, zero_outs = [], [], [], []
        for alloc in nc.m.functions[0].allocations:
            if not isinstance(alloc, mybir.MemoryLocationSet):
                continue
            name = alloc.memorylocations[0].name
            if alloc.kind == "ExternalInput":
                if name != partition_name:
                    in_names.append(name)
            elif alloc.kind == "ExternalOutput":
                shape = tuple(alloc.tensor_shape)
                dtype = mybir.dt.np(alloc.dtype)
                out_names.append(name)
                out_avals.append(jax.core.ShapedArray(shape, dtype))
                zero_outs.append(np.zeros(shape, dtype))
        self.in_names, self.out_names = in_names, out_names
        self.out_avals, self.zero_outs = out_avals, zero_outs
        n_params = len(in_names)
        all_in_names = list(in_names) + list(out_names)
        if partition_name is not None:
            all_in_names.append(partition_name)

        def _body(*args):
            ins = list(args[:n_params])
            cur_zeros = list(args[n_params:n_params + len(out_names)])
            extra = ([bass2jax.partition_id_tensor()]
                     if partition_name is not None else [])
            outs = tuple(cur_zeros)
            for _ in range(n_inner):
                # chain outputs into the next call's output buffers: keeps
                # every invocation live (no CSE/DCE) and is a no-op since
                # the kernel fully overwrites its outputs
                outs = bass2jax._bass_exec_p.bind(
                    *ins, *outs, *extra,
                    out_avals=tuple(out_avals),
                    in_names=tuple(all_in_names),
                    out_names=tuple(out_names),
                    lowering_input_output_aliases=(),
                    sim_require_finite=True,
                    sim_require_nnan=True,
                    nc=nc,
                )
            return tuple(outs)

        devices = jax.devices()[:n_cores]
        assert len(devices) == n_cores
        self.mesh = Mesh(np.asarray(devices), ("core",))
        self.pspec = PartitionSpec("core")
        n_outs = len(out_names)
        in_specs = (self.pspec,) * (n_params + n_outs)
        out_specs = (self.pspec,) * n_outs
        donate = tuple(range(n_params, n_params + n_outs))
        self.fn = jax.jit(
            shard_map(_body, mesh=self.mesh, in_specs=in_specs,
                      out_specs=out_specs, check_rep=False),
            donate_argnums=donate, keep_unused=True)

    def concat_inputs(self, in_maps):
        return [np.concatenate([np.asarray(in_maps[c][n])
                                for c in range(self.n_cores)], axis=0)
                for n in self.in_names]

    def device_put(self, concat_in):
        from jax.sharding import NamedSharding
        sh = NamedSharding(self.mesh, self.pspec)
        return [self.jax.device_put(a, sh) for a in concat_in]

    def run_dev(self, dev_args):
        zeros = [np.zeros((self.n_cores * z.shape[0], *z.shape[1:]), z.dtype)
                 for z in self.zero_outs]
        out = self.fn(*dev_args, *zeros)
        return self.jax.block_until_ready(out)

    def run(self, in_maps):
        out_arrs = self.run_dev(self.device_put(self.concat_inputs(in_maps)))
        return [
            {n: np.asarray(out_arrs[i]).reshape(
                self.n_cores, *self.out_avals[i].shape)[c]
             for i, n in enumerate(self.out_names)}
            for c in range(self.n_cores)
        ]


_CACHE = {}


def get_runner(variant=DEFAULT_VARIANT, reps=1, n_inner=1, **kw):
    key = (variant, reps, n_inner, tuple(sorted(kw.items())))
    if key not in _CACHE:
        if variant == "sq":
            nc = build_program_sq(reps=reps, **kw)
        else:
            nc = build_program(variant=variant, reps=reps, **kw)
        _CACHE[key] = Runner(nc, n_inner=n_inner)
    return _CACHE[key]


def kernel(states_x, x_target, molecule_id, num_molecules):
    runner = get_runner(DEFAULT_VARIANT)
    in_maps, V = host_prep(states_x, x_target, molecule_id, num_molecules,
                           variant=DEFAULT_VARIANT)
    results = runner.run(in_maps)
    return combine(results, V)

